# revision 1
# baseline (speedup 1.0000x reference)
"""Trainium2 Bass kernel for the seasonal-decomposition block.

Math: for each season s, circ_s = real(F_s^H diag(d_s) F_s) with F_s the s-th
diagonal LxL block of the normalized N=L*S DFT matrix. Expanding,
    circ_s[a, b] = (1/N) * sum_j d_s[j] * cos(2*pi*(s*L+j)*(a-b)/N)
depends only on a-b: a symmetric Toeplitz matrix whose first column
c_s(t) is computed on host with one length-N FFT. Every 128x128 block of
circ_s is a contiguous column slice of the skewed buffer
    E2r_s[p, m] = c_s(|2047 + p - m|)   (shape [128, 4096], 2 MB fp32)
so the LxL matrix is never materialized; the PE reads stationary operands
straight out of an 8 MB SBUF-resident E2r.

The recurrence  x_rem <- x_rem - tanh(x_rem @ circ_s)  runs in transposed
layout (positions on partitions, rows on the free axis) so no per-season
transposes are needed:  out[b, r] = sum_a circ[a, b] * xT[a, r]  via
matmul(lhsT=circ_block, rhs=xT_chunk).  The trailing avg-pool trend is two
banded matmuls per 128-chunk with three tiny host-built band matrices.
The output is accumulated as sum_s tanh_s + trend (never x - x_rem, which
would lose precision to cancellation).

Sharding: pure data-parallel over the B*C = 2048 rows, 256 rows per core,
8 cores, no collectives. Matmuls run in float32r (full PE rate, ~1.6e-4
relative error) with fp32 PSUM accumulation.
"""

import sys

sys.path.insert(0, "/opt/trn_rl_repo")

import numpy as np

import concourse.mybir as mybir
import concourse.tile as tile
from concourse import bacc
from concourse.bass_utils import run_bass_kernel_spmd

L = 2048
S = 4
NFULL = L * S
KER = 25
B, C = 64, 32
NCORES = 8
ROWS = B * C          # 2048
RPC = ROWS // NCORES  # 256 rows per core
NCHUNK = L // 128     # 16

_f32 = mybir.dt.float32
_f32r = mybir.dt.float32r


def _build_tband():
    """Three [128,128] band blocks of the avg-pool matrix T (trend = T.T @ x)."""
    u = np.arange(128)[:, None]
    t = np.arange(128)[None, :]
    diag = ((t - u >= 0) & (t - u <= KER - 1)).astype(np.float32) / KER
    sub = ((u - t) >= 128 - (KER - 1)).astype(np.float32) / KER
    t00 = diag.copy()
    t00[0, :] += np.maximum(0, (KER - 1) - np.arange(128)).astype(np.float32) / KER
    return np.ascontiguousarray(np.stack([t00, diag, sub], axis=1))  # [128, 3, 128]


_TBAND = _build_tband()
_E2R_IDX = np.clip(np.abs(2047 + np.arange(128)[:, None] - np.arange(4096)[None, :]), 0, L - 1)


def _circ_cols(diagonals):
    """First columns c_s(t), t = 0..L-1, of each season's Toeplitz circ_s."""
    d = np.zeros((S, NFULL), dtype=np.float64)
    d[:, :L] = np.asarray(diagonals, dtype=np.float64)
    F = np.fft.fft(d, axis=1)  # F[s,k] = sum_j d_j e^{-2pi i jk/N}
    t = np.arange(L)
    ph = np.exp((2j * np.pi / NFULL) * (np.arange(S)[:, None] * L * t[None, :]))
    return ((ph * np.conj(F[:, :L])).real / NFULL).astype(np.float32)  # [S, L]


def _emit_body(nc, pools, xr_d, e2l_d, e2rr_d, tb_d, out_d, mmdt=_f32r):
    constp, xrp, corrp, workp, psum_a, psum_t = pools
    tanh_f = mybir.ActivationFunctionType.Tanh

    # Prologue DMA order follows first use: the opening accumulation chain
    # needs x quarter 0 + the hi half of season-0 weights, then consumes x
    # quarters and weight pieces alternately.
    x0t = [constp.tile([128, 4, RPC], mmdt, tag=f"x0_{k}", name=f"x0_{k}") for k in range(4)]
    e2lh_sb, e2ll_sb, e2rr_sb = [], [], []
    for s in range(S):
        e2lh_sb.append(constp.tile([128, 1024], mmdt, tag=f"e2lh{s}", name=f"e2lh{s}"))
        e2ll_sb.append(constp.tile([128, 1024], mmdt, tag=f"e2ll{s}", name=f"e2ll{s}"))
        e2rr_sb.append(constp.tile([128, 1920], mmdt, tag=f"e2rr{s}", name=f"e2rr{s}"))
    nc.sync.dma_start(x0t[0][:], xr_d[0])
    nc.sync.dma_start(e2lh_sb[0][:], e2l_d[0][:, 1024:])
    nc.sync.dma_start(x0t[1][:], xr_d[1])
    nc.sync.dma_start(e2ll_sb[0][:], e2l_d[0][:, :1024])
    nc.sync.dma_start(x0t[2][:], xr_d[2])
    nc.sync.dma_start(x0t[3][:], xr_d[3])
    nc.sync.dma_start(e2rr_sb[0][:], e2rr_d[0])
    for s in range(1, S):
        nc.sync.dma_start(e2lh_sb[s][:], e2l_d[s][:, 1024:])
        nc.sync.dma_start(e2ll_sb[s][:], e2l_d[s][:, :1024])
        nc.sync.dma_start(e2rr_sb[s][:], e2rr_d[s])
    xr_cur = [x0t[a // 4][:, a % 4, :] for a in range(NCHUNK)]
    tb_sb = constp.tile([128, 3, 128], mmdt, tag="tb")
    nc.sync.dma_start(tb_sb[:], tb_d[:])

    def circ_block(s, a, b):
        d = a - b
        if 0 <= d <= 7:
            return e2lh_sb[s][:, 896 - 128 * d : 1024 - 128 * d]
        if d >= 8:
            return e2ll_sb[s][:, 1920 - 128 * d : 2048 - 128 * d]
        return e2rr_sb[s][:, -128 * (d + 1) : -128 * d]

    corr = [corrp.tile([128, RPC], _f32, tag=f"corr{b}", name=f"corr{b}") for b in range(NCHUNK)]
    big_ob = constp.tile([128, NCHUNK, RPC], _f32, tag="bigob")

    def emit_trend(j, xr3):
        tps = psum_t.tile([128, RPC], _f32, tag="acc" if psum_t is psum_a else "tps", name=f"tps{j}")
        if j == 0:
            nc.tensor.matmul(tps[:], tb_sb[:, 0, :], xr3[0], start=True, stop=True)
        else:
            nc.tensor.matmul(tps[:], tb_sb[:, 2, :], xr3[j - 1], start=True, stop=False)
            nc.tensor.matmul(tps[:], tb_sb[:, 1, :], xr3[j], start=False, stop=True)
        nc.vector.tensor_add(out=big_ob[:, j, :], in0=corr[j][:], in1=tps[:])
        if j % 4 == 3:
            q = j // 4
            nc.sync.dma_start(out_d[:, 4 * q : 4 * q + 4, :], big_ob[:, 4 * q : 4 * q + 4, :])

    for s in range(S):
        xr_next = [xrp.tile([128, RPC], mmdt, tag=f"xr{b}", name=f"xr{s}_{b}") for b in range(NCHUNK)]
        for b in range(NCHUNK):
            acc = psum_a.tile([128, RPC], _f32, tag="acc")
            # Chain order a = b..15 then 0..b-1: the a >= b blocks live in the
            # left weight halves, which arrive first; the a < b blocks (right
            # half) come last so season 0 never stalls on the e2rr DMA.
            a_order = list(range(b, NCHUNK)) + list(range(b))
            for i, a in enumerate(a_order):
                nc.tensor.matmul(
                    acc[:],
                    circ_block(s, a, b),
                    xr_cur[a],
                    start=(i == 0),
                    stop=(i == NCHUNK - 1),
                )
            if s == 0:
                nc.scalar.activation(corr[b][:], acc[:], tanh_f)
                nc.vector.tensor_sub(out=xr_next[b][:], in0=xr_cur[b], in1=corr[b][:])
            else:
                tmp = workp.tile([128, RPC], _f32, tag="tanh")
                nc.scalar.activation(tmp[:], acc[:], tanh_f)
                nc.vector.tensor_add(out=corr[b][:], in0=corr[b][:], in1=tmp[:])
                nc.vector.tensor_sub(out=xr_next[b][:], in0=xr_cur[b], in1=tmp[:])
            # Interleave trend chunks two groups behind season 3 so the PE
            # never waits on the DVE updates they read.
            if s == S - 1 and b >= 2:
                emit_trend(b - 2, xr_next)
        xr_cur = [t_[:] for t_ in xr_next]

    emit_trend(NCHUNK - 2, xr_cur)
    emit_trend(NCHUNK - 1, xr_cur)


def build_nc(reps=1, acc_bufs=6, merge_tps=True, mmdt=_f32r):
    nc = bacc.Bacc("TRN2", target_bir_lowering=False, debug=False)
    xr_d = nc.dram_tensor("xr", [4, 128, 4, RPC], mmdt, kind="ExternalInput")
    e2l_d = nc.dram_tensor("e2l", [S, 128, 2048], mmdt, kind="ExternalInput")
    e2rr_d = nc.dram_tensor("e2rr", [S, 128, 1920], mmdt, kind="ExternalInput")
    tb_d = nc.dram_tensor("tb", [128, 3, 128], mmdt, kind="ExternalInput")
    out_d = nc.dram_tensor("out", [128, NCHUNK, RPC], _f32, kind="ExternalOutput")

    with tile.TileContext(nc) as tc:
        with (
            tc.tile_pool(name="const", bufs=1) as constp,
            tc.tile_pool(name="xrp", bufs=2) as xrp,
            tc.tile_pool(name="corrp", bufs=1) as corrp,
            tc.tile_pool(name="work", bufs=4) as workp,
            tc.tile_pool(name="psum_a", bufs=acc_bufs, space="PSUM") as psum_a,
            tc.tile_pool(name="psum_t", bufs=2, space="PSUM") as psum_t,
        ):
            pools = (constp, xrp, corrp, workp, psum_a,
                     psum_a if merge_tps else psum_t)
            if reps == 1:
                _emit_body(nc, pools, xr_d, e2l_d, e2rr_d, tb_d, out_d, mmdt)
            else:
                with tc.For_i(0, reps, 1, staggered_reset=True,
                              hint_engines=(mybir.EngineType.PE,)):
                    _emit_body(nc, pools, xr_d, e2l_d, e2rr_d, tb_d, out_d, mmdt)
    nc.compile()
    return nc


_NC_CACHE = {}


def _get_nc(reps=1):
    if reps not in _NC_CACHE:
        _NC_CACHE[reps] = build_nc(reps)
    return _NC_CACHE[reps]


def make_in_maps(x, diagonals, np_dt=np.float32):
    c = _circ_cols(diagonals)
    e2r = c[:, _E2R_IDX]  # [S, 128, 4096]
    e2l = np.ascontiguousarray(e2r[:, :, 127:2175]).astype(np_dt)
    e2rr = np.ascontiguousarray(e2r[:, :, 2175:4095]).astype(np_dt)
    xT = np.asarray(x, dtype=np.float32).reshape(ROWS, L).T  # [L, ROWS] view
    in_maps = []
    for i in range(NCORES):
        xs = np.ascontiguousarray(xT[:, i * RPC : (i + 1) * RPC])
        xs = xs.reshape(NCHUNK, 128, RPC).transpose(1, 0, 2)  # [128, 16, RPC]
        xs = np.ascontiguousarray(xs.reshape(128, 4, 4, RPC).transpose(1, 0, 2, 3))
        in_maps.append({"xr": xs.astype(np_dt), "e2l": e2l, "e2rr": e2rr, "tb": _TBAND.astype(np_dt)})
    return in_maps


def gather_out(results):
    parts = []
    for r in results:
        o = r["out"]  # [128, NCHUNK, RPC]
        parts.append(np.ascontiguousarray(o.transpose(1, 0, 2)).reshape(L, RPC))
    outT = np.concatenate(parts, axis=1)  # [L, ROWS]
    return np.ascontiguousarray(outT.T).reshape(B, C, L).astype(np.float32)


def kernel(x, diagonals):
    x = np.asarray(x, dtype=np.float32)
    assert x.shape == (B, C, L) and np.asarray(diagonals).shape == (S, L)
    nc = _get_nc(1)
    in_maps = make_in_maps(x, diagonals)
    last_err = None
    for attempt in range(3):
        try:
            res = run_bass_kernel_spmd(nc, in_maps, core_ids=list(range(NCORES)))
            return gather_out(res.results)
        except Exception as ex:  # transient device errors (e.g. NRT_EXEC_UNIT_UNRECOVERABLE)
            last_err = ex
            import time as _time

            _time.sleep(2.0 * (attempt + 1))
    raise last_err



# revision 2
# speedup vs baseline: 1.2116x; 1.2116x over previous
"""Trainium2 Bass kernel for the seasonal-decomposition block (fp8 DoubleRow).

Math: for each season s, circ_s = real(F_s^H diag(d_s) F_s) is a symmetric
Toeplitz matrix whose first column c_s(t) is computed on host with one
length-N FFT. Every 128x128 block of circ_s is a contiguous 128-column
slice of the skewed buffer E2F_s[p, m] = c_s(|1920 + p - m|) ([128, 3968]),
so the LxL matrix is never materialized.

The recurrence  x_rem <- x_rem - tanh(x_rem @ circ_s)  runs in transposed
layout (positions on partitions, rows on the free axis). Matmuls use fp8
(e4m3) operands with MatmulPerfMode.DoubleRow: each instruction contracts
two adjacent 128-blocks at double rate. Adjacent position-chunks are
pair-swapped in the x layout (position j holds logical chunk j^1) so that
a single [128, 2, RPC] access pattern pairs with two adjacent weight
blocks of the skew buffer, which are contiguous by construction.

Precision: weights are scaled by 256 on host before fp8 quantization
(entries ~3e-3 would be subnormal otherwise) and descaled inside the tanh
activation (scale=1/256). The x_rem state is mastered in bf16; the fp8
replica fed to the PE is refreshed from the bf16 master every season
(gpsimd copy). tanh outputs and the corr accumulator are bf16; the
trailing avg-pool trend is computed from the bf16 master with bf16 band
matrices (fp8 bands would corrupt the replicate-pad coefficients). The
output is assembled as corr + trend in fp32. Emulated end-to-end rel_err
vs the fp64 reference: 1.46e-2 (gate: 2e-2).

Sharding: pure data-parallel over the B*C = 2048 rows, 256 rows per core,
8 cores, no collectives.
"""

import sys

sys.path.insert(0, "/opt/trn_rl_repo")

import numpy as np
import ml_dtypes

import concourse.mybir as mybir
import concourse.tile as tile
from concourse import bacc
from concourse.bass_utils import run_bass_kernel_spmd

L = 2048
S = 4
NFULL = L * S
KER = 25
B, C = 64, 32
NCORES = 8
ROWS = B * C          # 2048
RPC = ROWS // NCORES  # 256 rows per core
NCHUNK = L // 128     # 16
WSCALE = 256.0

_f32 = mybir.dt.float32
_f8 = mybir.dt.float8e4
_bf16 = mybir.dt.bfloat16
_np_f8 = ml_dtypes.float8_e4m3
_np_bf16 = ml_dtypes.bfloat16
_DR = mybir.MatmulPerfMode.DoubleRow


def _build_tband():
    """Three [128,128] band blocks of the avg-pool matrix T (trend = T.T @ x)."""
    u = np.arange(128)[:, None]
    t = np.arange(128)[None, :]
    diag = ((t - u >= 0) & (t - u <= KER - 1)).astype(np.float32) / KER
    sub = ((u - t) >= 128 - (KER - 1)).astype(np.float32) / KER
    t00 = diag.copy()
    t00[0, :] += np.maximum(0, (KER - 1) - np.arange(128)).astype(np.float32) / KER
    return np.ascontiguousarray(np.stack([t00, diag, sub], axis=1))  # [128, 3, 128]


_TBAND = _build_tband()
# skew index: E2F[p, m] = c(|1920 + p - m|), block(d) at cols [1920-128d, 2048-128d)
_E2F_IDX = np.abs(1920 + np.arange(128)[:, None] - np.arange(31 * 128)[None, :])
# pair-swap permutation: position j holds logical chunk j^1
_PERM = np.arange(NCHUNK) ^ 1


def _circ_cols(diagonals):
    """First columns c_s(t), t = 0..L-1, of each season's Toeplitz circ_s."""
    d = np.zeros((S, NFULL), dtype=np.float64)
    d[:, :L] = np.asarray(diagonals, dtype=np.float64)
    F = np.fft.fft(d, axis=1)
    t = np.arange(L)
    ph = np.exp((2j * np.pi / NFULL) * (np.arange(S)[:, None] * L * t[None, :]))
    return ((ph * np.conj(F[:, :L])).real / NFULL).astype(np.float32)  # [S, L]


def _emit_body(nc, pools, xr8_d, xbf_d, e2_d, tb_d, out_d):
    constp, xrp, workp, psum_a, psum_t = pools
    tanh_f = mybir.ActivationFunctionType.Tanh

    # ---- SBUF tiles + prologue DMA (ordered by first use) ----
    e2_sb = [constp.tile([128, 31, 128], _f8, tag=f"e2_{s}", name=f"e2_{s}") for s in range(S)]
    xr8_0 = constp.tile([128, NCHUNK, RPC], _f8, tag="xr8_0", name="xr8_0")
    xbf_0 = constp.tile([128, NCHUNK, RPC], _bf16, tag="xbf_0", name="xbf_0")
    tb_sb = constp.tile([128, 3, 128], _bf16, tag="tb")
    corr = constp.tile([128, NCHUNK, RPC], _bf16, tag="corr", name="corr")
    big_ob = constp.tile([128, NCHUNK, RPC], _f32, tag="bigob")

    # season-0 chains consume weights in ascending k; x first
    nc.sync.dma_start(xr8_0[:, 0:8, :], xr8_d[:, 0:8, :])
    nc.sync.dma_start(e2_sb[0][:, 0:16, :], e2_d[0][:, 0:16, :])
    nc.sync.dma_start(xr8_0[:, 8:16, :], xr8_d[:, 8:16, :])
    nc.sync.dma_start(e2_sb[0][:, 16:31, :], e2_d[0][:, 16:31, :])
    for s in range(1, S):
        nc.sync.dma_start(e2_sb[s][:], e2_d[s])
    nc.sync.dma_start(xbf_0[:], xbf_d[:])
    nc.sync.dma_start(tb_sb[:], tb_d[:])

    xr8_cur = xr8_0
    xbf_cur = xbf_0

    for s in range(S):
        last = s == S - 1
        xr8_nxt = None if last else xrp.tile(
            [128, NCHUNK, RPC], _f8, tag="xr8n", name=f"xr8n{s}")
        xbf_nxt = xrp.tile([128, NCHUNK, RPC], _bf16, tag="xbfn", name=f"xbfn{s}")
        t_pair = [None] * (NCHUNK // 2)
        for b in range(NCHUNK):
            acc = psum_a.tile([128, RPC], _f32, tag="acc")
            # season 0: ascending k (weight DMA arrival order);
            # later seasons: ascending pair (x written pair-by-pair)
            p_order = range(7, -1, -1) if s == 0 else range(8)
            for i, p in enumerate(p_order):
                k = b + 14 - 2 * p
                nc.tensor.matmul(
                    acc[:],
                    e2_sb[s][:, k : k + 2, :],
                    xr8_cur[:, 2 * p : 2 * p + 2, :],
                    start=(i == 0),
                    stop=(i == 7),
                    perf_mode=_DR,
                )
            q = b // 2
            if t_pair[q] is None:
                t_pair[q] = workp.tile([128, 2, RPC], _bf16, tag="tp", name=f"tp{s}_{q}")
            # chunk b lives at position b^1; within the pair that's 1-b%2
            nc.scalar.activation(t_pair[q][:, 1 - b % 2, :], acc[:], tanh_f,
                                 scale=1.0 / WSCALE)
            if b % 2 == 1:
                sl = slice(2 * q, 2 * q + 2)
                # bf16 master update + corr accumulate + fp8 replica refresh
                nc.vector.tensor_sub(out=xbf_nxt[:, sl, :], in0=xbf_cur[:, sl, :],
                                     in1=t_pair[q][:])
                if s == 0:
                    nc.gpsimd.tensor_copy(out=corr[:, sl, :], in_=t_pair[q][:])
                else:
                    nc.gpsimd.tensor_add(out=corr[:, sl, :], in0=corr[:, sl, :],
                                         in1=t_pair[q][:])
                if not last:
                    nc.gpsimd.tensor_copy(out=xr8_nxt[:, sl, :], in_=xbf_nxt[:, sl, :])
        xr8_cur = xr8_nxt
        xbf_cur = xbf_nxt

    # ---- trend = T.T @ x_rem4 (bf16 master), out = corr + trend ----
    for j in range(NCHUNK):
        tps = psum_t.tile([128, RPC], _f32, tag="tps", name=f"tps{j}")
        if j == 0:
            nc.tensor.matmul(tps[:], tb_sb[:, 0, :], xbf_cur[:, 1, :],
                             start=True, stop=True)
        else:
            nc.tensor.matmul(tps[:], tb_sb[:, 2, :], xbf_cur[:, (j - 1) ^ 1, :],
                             start=True, stop=False)
            nc.tensor.matmul(tps[:], tb_sb[:, 1, :], xbf_cur[:, j ^ 1, :],
                             start=False, stop=True)
        nc.vector.tensor_add(out=big_ob[:, j, :], in0=corr[:, j ^ 1, :], in1=tps[:])
        if j % 4 == 3:
            qn = j // 4
            nc.sync.dma_start(out_d[:, 4 * qn : 4 * qn + 4, :],
                              big_ob[:, 4 * qn : 4 * qn + 4, :])


def build_nc(reps=1):
    nc = bacc.Bacc("TRN2", target_bir_lowering=False, debug=False)
    xr8_d = nc.dram_tensor("xr8", [128, NCHUNK, RPC], _f8, kind="ExternalInput")
    xbf_d = nc.dram_tensor("xbf", [128, NCHUNK, RPC], _bf16, kind="ExternalInput")
    e2_d = nc.dram_tensor("e2", [S, 128, 31, 128], _f8, kind="ExternalInput")
    tb_d = nc.dram_tensor("tb", [128, 3, 128], _bf16, kind="ExternalInput")
    out_d = nc.dram_tensor("out", [128, NCHUNK, RPC], _f32, kind="ExternalOutput")

    with tile.TileContext(nc) as tc:
        with (
            tc.tile_pool(name="const", bufs=1) as constp,
            tc.tile_pool(name="xrp", bufs=2) as xrp,
            tc.tile_pool(name="work", bufs=6) as workp,
            tc.tile_pool(name="psum_a", bufs=6, space="PSUM") as psum_a,
            tc.tile_pool(name="psum_t", bufs=2, space="PSUM") as psum_t,
        ):
            pools = (constp, xrp, workp, psum_a, psum_t)
            if reps == 1:
                _emit_body(nc, pools, xr8_d, xbf_d, e2_d, tb_d, out_d)
            else:
                with tc.For_i(0, reps, 1, staggered_reset=True,
                              hint_engines=(mybir.EngineType.PE,)):
                    _emit_body(nc, pools, xr8_d, xbf_d, e2_d, tb_d, out_d)
    nc.compile()
    return nc


_NC_CACHE = {}


def _get_nc(reps=1):
    if reps not in _NC_CACHE:
        _NC_CACHE[reps] = build_nc(reps)
    return _NC_CACHE[reps]


def make_in_maps(x, diagonals):
    c = _circ_cols(diagonals)
    e2 = (c * WSCALE)[:, _E2F_IDX].astype(_np_f8).reshape(S, 128, 31, 128)
    tb = _TBAND.astype(_np_bf16)
    xT = np.asarray(x, dtype=np.float32).reshape(ROWS, L).T  # [L, ROWS]
    in_maps = []
    for i in range(NCORES):
        xs = xT[:, i * RPC : (i + 1) * RPC].reshape(NCHUNK, 128, RPC)
        xs = np.ascontiguousarray(xs[_PERM].transpose(1, 0, 2))  # [128, 16, RPC]
        in_maps.append({
            "xr8": xs.astype(_np_f8),
            "xbf": xs.astype(_np_bf16),
            "e2": e2,
            "tb": tb,
        })
    return in_maps


def gather_out(results):
    parts = []
    for r in results:
        o = r["out"]  # [128, NCHUNK, RPC]
        parts.append(np.ascontiguousarray(o.transpose(1, 0, 2)).reshape(L, RPC))
    outT = np.concatenate(parts, axis=1)  # [L, ROWS]
    return np.ascontiguousarray(outT.T).reshape(B, C, L).astype(np.float32)


def kernel(x, diagonals):
    x = np.asarray(x, dtype=np.float32)
    assert x.shape == (B, C, L) and np.asarray(diagonals).shape == (S, L)
    nc = _get_nc(1)
    in_maps = make_in_maps(x, diagonals)
    last_err = None
    for attempt in range(3):
        try:
            res = run_bass_kernel_spmd(nc, in_maps, core_ids=list(range(NCORES)))
            return gather_out(res.results)
        except Exception as ex:  # transient device errors
            last_err = ex
            import time as _time

            _time.sleep(2.0 * (attempt + 1))
    raise last_err


# revision 3
# speedup vs baseline: 1.6704x; 1.3786x over previous
"""Trainium2 Bass kernel for the seasonal-decomposition block (fp8 DoubleRow).

Math: for each season s, circ_s = real(F_s^H diag(d_s) F_s) is a symmetric
Toeplitz matrix whose first column c_s(t) is computed on host with one
length-N FFT. Every 128x128 block of circ_s is a contiguous 128-column
slice of the skewed buffer E2F_s[p, m] = c_s(|1920 + p - m|) ([128, 3968]),
so the LxL matrix is never materialized.

The recurrence  x_rem <- x_rem - tanh(x_rem @ circ_s)  runs in transposed
layout (positions on partitions, rows on the free axis). Matmuls use fp8
(e4m3) operands with MatmulPerfMode.DoubleRow: each instruction contracts
two adjacent 128-blocks at double rate. Adjacent position-chunks are
pair-swapped in the x layout (position j holds logical chunk j^1) so a
single [128, 2, RPC] access pattern pairs with two adjacent weight blocks
of the skew buffer, which are contiguous by construction.

State: x_rem is never materialized. Using x_rem_s = x - corr_s (with
corr_s = sum_{r<s} tanh_r, kept in bf16), the fp8 PE operand for the next
season is one fused DVE op  xr8 = fp8(x_bf - corr)  per chunk-pair, and
the trailing avg-pool trend is  T.T @ x_bf - T.T @ corr  accumulated in
one PSUM group via positive and negated bf16 band matrices (fp8 bands
would corrupt the replicate-pad coefficients). Weights are scaled by 256
on host before fp8 quantization (entries ~3e-3 would be subnormal) and
descaled inside the tanh activation (scale=1/256). Emulated end-to-end
rel_err vs the fp64 reference: 1.20e-2 (gate: 2e-2).

Sharding: pure data-parallel over the B*C = 2048 rows, 256 rows per core,
8 cores, no collectives.
"""

import sys

sys.path.insert(0, "/opt/trn_rl_repo")

import numpy as np
import ml_dtypes

import concourse.mybir as mybir
import concourse.tile as tile
from concourse import bacc
from concourse.bass_utils import run_bass_kernel_spmd

L = 2048
S = 4
NFULL = L * S
KER = 25
B, C = 64, 32
NCORES = 8
ROWS = B * C          # 2048
RPC = ROWS // NCORES  # 256 rows per core
NCHUNK = L // 128     # 16
WSCALE = 256.0

_f32 = mybir.dt.float32
_f8 = mybir.dt.float8e4
_bf16 = mybir.dt.bfloat16
_np_f8 = ml_dtypes.float8_e4m3
_np_bf16 = ml_dtypes.bfloat16
_DR = mybir.MatmulPerfMode.DoubleRow


def _build_tband():
    """Three [128,128] band blocks of the avg-pool matrix T (trend = T.T @ x)."""
    u = np.arange(128)[:, None]
    t = np.arange(128)[None, :]
    diag = ((t - u >= 0) & (t - u <= KER - 1)).astype(np.float32) / KER
    sub = ((u - t) >= 128 - (KER - 1)).astype(np.float32) / KER
    t00 = diag.copy()
    t00[0, :] += np.maximum(0, (KER - 1) - np.arange(128)).astype(np.float32) / KER
    return np.ascontiguousarray(np.stack([t00, diag, sub], axis=1))  # [128, 3, 128]


_TBAND = _build_tband()
# skew index: E2F[p, m] = c(|1920 + p - m|), block(d) at cols [1920-128d, 2048-128d)
_E2F_IDX = np.abs(1920 + np.arange(128)[:, None] - np.arange(31 * 128)[None, :])
# pair-swap permutation: position j holds logical chunk j^1
_PERM = np.arange(NCHUNK) ^ 1


def _circ_cols(diagonals):
    """First columns c_s(t), t = 0..L-1, of each season's Toeplitz circ_s."""
    d = np.zeros((S, NFULL), dtype=np.float64)
    d[:, :L] = np.asarray(diagonals, dtype=np.float64)
    F = np.fft.fft(d, axis=1)
    t = np.arange(L)
    ph = np.exp((2j * np.pi / NFULL) * (np.arange(S)[:, None] * L * t[None, :]))
    return ((ph * np.conj(F[:, :L])).real / NFULL).astype(np.float32)  # [S, L]


def _emit_body(nc, pools, xr8_d, xbf_d, e2_d, tb_d, out_d):
    constp, xrp, workp, psum_a, psum_t = pools
    tanh_f = mybir.ActivationFunctionType.Tanh

    # ---- SBUF tiles + prologue DMA (ordered by first use) ----
    e2_sb = [constp.tile([128, 31, 128], _f8, tag=f"e2_{s}", name=f"e2_{s}") for s in range(S)]
    xr8_0 = constp.tile([128, NCHUNK, RPC], _f8, tag="xr8_0", name="xr8_0")
    xbf_0 = constp.tile([128, NCHUNK, RPC], _bf16, tag="xbf_0", name="xbf_0")
    tb_sb = constp.tile([128, 6, 128], _bf16, tag="tb")
    corr = constp.tile([128, NCHUNK, RPC], _bf16, tag="corr", name="corr")
    big_ob = constp.tile([128, NCHUNK, RPC], _f32, tag="bigob")

    # season-0 chains consume weights in ascending k; x first
    nc.sync.dma_start(xr8_0[:, 0:8, :], xr8_d[:, 0:8, :])
    nc.sync.dma_start(e2_sb[0][:, 0:16, :], e2_d[0][:, 0:16, :])
    nc.sync.dma_start(xr8_0[:, 8:16, :], xr8_d[:, 8:16, :])
    nc.sync.dma_start(e2_sb[0][:, 16:31, :], e2_d[0][:, 16:31, :])
    nc.sync.dma_start(xbf_0[:], xbf_d[:])
    for s in range(1, S):
        nc.sync.dma_start(e2_sb[s][:], e2_d[s])
    nc.sync.dma_start(tb_sb[:], tb_d[:])

    xr8_cur = xr8_0

    for s in range(S):
        last = s == S - 1
        xr8_nxt = None if last else xrp.tile(
            [128, NCHUNK, RPC], _f8, tag="xr8n", name=f"xr8n{s}")
        t_pair = [None] * (NCHUNK // 2)
        for b in range(NCHUNK):
            acc = psum_a.tile([128, RPC], _f32, tag="acc")
            # season 0: ascending k (weight DMA arrival order);
            # later seasons: ascending pair (x written pair-by-pair)
            p_order = range(7, -1, -1) if s == 0 else range(8)
            for i, p in enumerate(p_order):
                k = b + 14 - 2 * p
                nc.tensor.matmul(
                    acc[:],
                    e2_sb[s][:, k : k + 2, :],
                    xr8_cur[:, 2 * p : 2 * p + 2, :],
                    start=(i == 0),
                    stop=(i == 7),
                    perf_mode=_DR,
                )
            q = b // 2
            if t_pair[q] is None:
                t_pair[q] = workp.tile([128, 2, RPC], _bf16, tag="tp", name=f"tp{s}_{q}")
            # chunk b lives at position b^1; within the pair that's 1-b%2
            nc.scalar.activation(t_pair[q][:, 1 - b % 2, :], acc[:], tanh_f,
                                 scale=1.0 / WSCALE)
            if b % 2 == 1:
                sl = slice(2 * q, 2 * q + 2)
                # corr accumulate (gpsimd), then fused  xr8 = fp8(x - corr)  (DVE)
                if s == 0:
                    nc.gpsimd.tensor_copy(out=corr[:, sl, :], in_=t_pair[q][:])
                else:
                    nc.gpsimd.tensor_add(out=corr[:, sl, :], in0=corr[:, sl, :],
                                         in1=t_pair[q][:])
                if not last:
                    nc.vector.tensor_sub(out=xr8_nxt[:, sl, :], in0=xbf_0[:, sl, :],
                                         in1=corr[:, sl, :])
        xr8_cur = xr8_nxt

    # ---- trend = T.T @ (x - corr) in one PSUM group; out = corr + trend ----
    for j in range(NCHUNK):
        tps = psum_t.tile([128, RPC], _f32, tag="tps", name=f"tps{j}")
        if j == 0:
            nc.tensor.matmul(tps[:], tb_sb[:, 0, :], xbf_0[:, 1, :],
                             start=True, stop=False)
            nc.tensor.matmul(tps[:], tb_sb[:, 3, :], corr[:, 1, :],
                             start=False, stop=True)
        else:
            nc.tensor.matmul(tps[:], tb_sb[:, 2, :], xbf_0[:, (j - 1) ^ 1, :],
                             start=True, stop=False)
            nc.tensor.matmul(tps[:], tb_sb[:, 1, :], xbf_0[:, j ^ 1, :],
                             start=False, stop=False)
            nc.tensor.matmul(tps[:], tb_sb[:, 5, :], corr[:, (j - 1) ^ 1, :],
                             start=False, stop=False)
            nc.tensor.matmul(tps[:], tb_sb[:, 4, :], corr[:, j ^ 1, :],
                             start=False, stop=True)
        nc.vector.tensor_add(out=big_ob[:, j, :], in0=corr[:, j ^ 1, :], in1=tps[:])
        if j % 4 == 3:
            qn = j // 4
            nc.sync.dma_start(out_d[:, 4 * qn : 4 * qn + 4, :],
                              big_ob[:, 4 * qn : 4 * qn + 4, :])


def build_nc(reps=1):
    nc = bacc.Bacc("TRN2", target_bir_lowering=False, debug=False)
    xr8_d = nc.dram_tensor("xr8", [128, NCHUNK, RPC], _f8, kind="ExternalInput")
    xbf_d = nc.dram_tensor("xbf", [128, NCHUNK, RPC], _bf16, kind="ExternalInput")
    e2_d = nc.dram_tensor("e2", [S, 128, 31, 128], _f8, kind="ExternalInput")
    tb_d = nc.dram_tensor("tb", [128, 6, 128], _bf16, kind="ExternalInput")
    out_d = nc.dram_tensor("out", [128, NCHUNK, RPC], _f32, kind="ExternalOutput")

    with tile.TileContext(nc) as tc:
        with (
            tc.tile_pool(name="const", bufs=1) as constp,
            tc.tile_pool(name="xrp", bufs=2) as xrp,
            tc.tile_pool(name="work", bufs=6) as workp,
            tc.tile_pool(name="psum_a", bufs=6, space="PSUM") as psum_a,
            tc.tile_pool(name="psum_t", bufs=2, space="PSUM") as psum_t,
        ):
            pools = (constp, xrp, workp, psum_a, psum_t)
            if reps == 1:
                _emit_body(nc, pools, xr8_d, xbf_d, e2_d, tb_d, out_d)
            else:
                with tc.For_i(0, reps, 1, staggered_reset=True,
                              hint_engines=(mybir.EngineType.PE,)):
                    _emit_body(nc, pools, xr8_d, xbf_d, e2_d, tb_d, out_d)
    nc.compile()
    return nc


_NC_CACHE = {}


def _get_nc(reps=1):
    if reps not in _NC_CACHE:
        _NC_CACHE[reps] = build_nc(reps)
    return _NC_CACHE[reps]


def make_in_maps(x, diagonals):
    c = _circ_cols(diagonals)
    e2 = (c * WSCALE)[:, _E2F_IDX].astype(_np_f8).reshape(S, 128, 31, 128)
    tb = np.concatenate([_TBAND, -_TBAND], axis=1).astype(_np_bf16)  # [128, 6, 128]
    xT = np.asarray(x, dtype=np.float32).reshape(ROWS, L).T  # [L, ROWS]
    in_maps = []
    for i in range(NCORES):
        xs = xT[:, i * RPC : (i + 1) * RPC].reshape(NCHUNK, 128, RPC)
        xs = np.ascontiguousarray(xs[_PERM].transpose(1, 0, 2))  # [128, 16, RPC]
        in_maps.append({
            "xr8": xs.astype(_np_f8),
            "xbf": xs.astype(_np_bf16),
            "e2": e2,
            "tb": tb,
        })
    return in_maps


def gather_out(results):
    parts = []
    for r in results:
        o = r["out"]  # [128, NCHUNK, RPC]
        parts.append(np.ascontiguousarray(o.transpose(1, 0, 2)).reshape(L, RPC))
    outT = np.concatenate(parts, axis=1)  # [L, ROWS]
    return np.ascontiguousarray(outT.T).reshape(B, C, L).astype(np.float32)


def kernel(x, diagonals):
    x = np.asarray(x, dtype=np.float32)
    assert x.shape == (B, C, L) and np.asarray(diagonals).shape == (S, L)
    nc = _get_nc(1)
    in_maps = make_in_maps(x, diagonals)
    last_err = None
    for attempt in range(3):
        try:
            res = run_bass_kernel_spmd(nc, in_maps, core_ids=list(range(NCORES)))
            return gather_out(res.results)
        except Exception as ex:  # transient device errors
            last_err = ex
            import time as _time

            _time.sleep(2.0 * (attempt + 1))
    raise last_err


# revision 7
# speedup vs baseline: 1.6795x; 1.0055x over previous
"""Trainium2 Bass kernel for the seasonal-decomposition block (fp8 DoubleRow).

Math: for each season s, circ_s = real(F_s^H diag(d_s) F_s) is a symmetric
Toeplitz matrix whose first column c_s(t) is computed on host with one
length-N FFT. Every 128x128 block of circ_s is a contiguous 128-column
slice of the skewed buffer E2F_s[p, m] = c_s(|1920 + p - m|) ([128, 3968]),
so the LxL matrix is never materialized.

The recurrence  x_rem <- x_rem - tanh(x_rem @ circ_s)  runs in transposed
layout (positions on partitions, rows on the free axis). Matmuls use fp8
(e4m3) operands with MatmulPerfMode.DoubleRow: each instruction contracts
two adjacent 128-blocks at double rate. Adjacent position-chunks are
pair-swapped in the x layout (position j holds logical chunk j^1) so a
single [128, 2, RPC] access pattern pairs with two adjacent weight blocks
of the skew buffer, which are contiguous by construction.

State: x_rem is never materialized. Using x_rem_s = x - corr_s (with
corr_s = sum_{r<s} tanh_r, kept in bf16), the fp8 PE operand for the next
season is one fused DVE op  xr8 = fp8(x_bf - corr)  per chunk-pair, and
the trailing avg-pool trend is  T.T @ x_bf - T.T @ corr  accumulated in
one PSUM group via positive and negated bf16 band matrices (fp8 bands
would corrupt the replicate-pad coefficients). Weights are scaled by 256
on host before fp8 quantization (entries ~3e-3 would be subnormal) and
descaled inside the tanh activation (scale=1/256). Emulated end-to-end
rel_err vs the fp64 reference: 1.20e-2 (gate: 2e-2).

Sharding: pure data-parallel over the B*C = 2048 rows, 256 rows per core,
8 cores, no collectives.
"""

import sys

sys.path.insert(0, "/opt/trn_rl_repo")

import numpy as np
import ml_dtypes

import concourse.mybir as mybir
import concourse.tile as tile
from concourse import bacc
from concourse.bass_utils import run_bass_kernel_spmd

L = 2048
S = 4
NFULL = L * S
KER = 25
B, C = 64, 32
NCORES = 8
ROWS = B * C          # 2048
RPC = ROWS // NCORES  # 256 rows per core
NCHUNK = L // 128     # 16
WSCALE = 256.0

_f32 = mybir.dt.float32
_f8 = mybir.dt.float8e4
_bf16 = mybir.dt.bfloat16
_np_f8 = ml_dtypes.float8_e4m3
_np_bf16 = ml_dtypes.bfloat16
_DR = mybir.MatmulPerfMode.DoubleRow


def _build_tband():
    """Three [128,128] band blocks of the avg-pool matrix T (trend = T.T @ x)."""
    u = np.arange(128)[:, None]
    t = np.arange(128)[None, :]
    diag = ((t - u >= 0) & (t - u <= KER - 1)).astype(np.float32) / KER
    sub = ((u - t) >= 128 - (KER - 1)).astype(np.float32) / KER
    t00 = diag.copy()
    t00[0, :] += np.maximum(0, (KER - 1) - np.arange(128)).astype(np.float32) / KER
    return np.ascontiguousarray(np.stack([t00, diag, sub], axis=1))  # [128, 3, 128]


_TBAND = _build_tband()
# skew index: E2F[p, m] = c(|1920 + p - m|), block(d) at cols [1920-128d, 2048-128d)
_E2F_IDX = np.abs(1920 + np.arange(128)[:, None] - np.arange(31 * 128)[None, :])
# pair-swap permutation: position j holds logical chunk j^1
_PERM = np.arange(NCHUNK) ^ 1


def _circ_cols(diagonals):
    """First columns c_s(t), t = 0..L-1, of each season's Toeplitz circ_s."""
    d = np.zeros((S, NFULL), dtype=np.float64)
    d[:, :L] = np.asarray(diagonals, dtype=np.float64)
    F = np.fft.fft(d, axis=1)
    t = np.arange(L)
    ph = np.exp((2j * np.pi / NFULL) * (np.arange(S)[:, None] * L * t[None, :]))
    return ((ph * np.conj(F[:, :L])).real / NFULL).astype(np.float32)  # [S, L]


def _emit_body(nc, pools, xr8_d, xbf_d, e2_d, tb_d, out_d):
    constp, xrp, workp, psum_a, psum_t = pools
    tanh_f = mybir.ActivationFunctionType.Tanh

    # ---- SBUF tiles + prologue DMA (ordered by first use) ----
    e2_sb = [constp.tile([128, 31, 128], _f8, tag=f"e2_{s}", name=f"e2_{s}") for s in range(S)]
    xr8_0 = constp.tile([128, NCHUNK, RPC], _f8, tag="xr8_0", name="xr8_0")
    xbf_0 = constp.tile([128, NCHUNK, RPC], _bf16, tag="xbf_0", name="xbf_0")
    tb_sb = constp.tile([128, 6, 128], _bf16, tag="tb")
    corr = constp.tile([128, NCHUNK, RPC], _bf16, tag="corr", name="corr")
    big_ob = constp.tile([128, NCHUNK, RPC], _f32, tag="bigob")

    # season-0 chain b=0 runs k ascending with rhs pair p=7 first: high x
    # positions and low-k weights must land first
    nc.sync.dma_start(xr8_0[:, 8:16, :], xr8_d[:, 8:16, :])
    nc.sync.dma_start(e2_sb[0][:, 0:6, :], e2_d[0][:, 0:6, :])
    nc.sync.dma_start(xr8_0[:, 0:8, :], xr8_d[:, 0:8, :])
    nc.sync.dma_start(e2_sb[0][:, 6:16, :], e2_d[0][:, 6:16, :])
    nc.sync.dma_start(e2_sb[0][:, 16:31, :], e2_d[0][:, 16:31, :])
    nc.sync.dma_start(xbf_0[:], xbf_d[:])
    for s in range(1, S):
        nc.sync.dma_start(e2_sb[s][:], e2_d[s])
    nc.sync.dma_start(tb_sb[:], tb_d[:])

    xr8_cur = xr8_0

    for s in range(S):
        last = s == S - 1
        xr8_nxt = None if last else xrp.tile(
            [128, NCHUNK, RPC], _f8, tag="xr8n", name=f"xr8n{s}")
        for b in range(NCHUNK):
            q = b // 2
            if b % 2 == 0:
                acc = psum_a.tile([128, 2, RPC], _f32, tag="acc")
            # season 0: ascending k (weight DMA arrival order);
            # later seasons: ascending pair (x written pair-by-pair)
            p_order = range(7, -1, -1) if s == 0 else range(8)
            for i, p in enumerate(p_order):
                k = b + 14 - 2 * p
                # chunk b lives at position b^1; within the pair that's 1-b%2
                nc.tensor.matmul(
                    acc[:, 1 - b % 2, :],
                    e2_sb[s][:, k : k + 2, :],
                    xr8_cur[:, 2 * p : 2 * p + 2, :],
                    start=(i == 0),
                    stop=(i == 7),
                    perf_mode=_DR,
                )
            if b % 2 == 1:
                sl = slice(2 * q, 2 * q + 2)
                t_pair = workp.tile([128, 2, RPC], _bf16, tag="tp", name=f"tp{s}_{q}")
                nc.scalar.activation(t_pair[:], acc[:], tanh_f, scale=1.0 / WSCALE)
                # corr accumulate (gpsimd), then fused  xr8 = fp8(x - corr)  (DVE)
                if s == 0:
                    nc.gpsimd.tensor_copy(out=corr[:, sl, :], in_=t_pair[:])
                else:
                    nc.gpsimd.tensor_add(out=corr[:, sl, :], in0=corr[:, sl, :],
                                         in1=t_pair[:])
                if not last:
                    nc.vector.tensor_sub(out=xr8_nxt[:, sl, :], in0=xbf_0[:, sl, :],
                                         in1=corr[:, sl, :])
        xr8_cur = xr8_nxt

    # ---- trend = T.T @ (x - corr) in one PSUM group; out = corr + trend ----
    # big_ob is kept in position order (chunk j^1 at index j); host unswaps.
    for j in range(NCHUNK):
        if j % 2 == 0:
            tps = psum_t.tile([128, 2, RPC], _f32, tag="tps", name=f"tps{j//2}")
        sl1 = 1 - j % 2  # position of logical chunk j within its pair
        if j == 0:
            nc.tensor.matmul(tps[:, sl1, :], tb_sb[:, 0, :], xbf_0[:, 1, :],
                             start=True, stop=False)
            nc.tensor.matmul(tps[:, sl1, :], tb_sb[:, 3, :], corr[:, 1, :],
                             start=False, stop=True)
        else:
            nc.tensor.matmul(tps[:, sl1, :], tb_sb[:, 2, :], xbf_0[:, (j - 1) ^ 1, :],
                             start=True, stop=False)
            nc.tensor.matmul(tps[:, sl1, :], tb_sb[:, 1, :], xbf_0[:, j ^ 1, :],
                             start=False, stop=False)
            nc.tensor.matmul(tps[:, sl1, :], tb_sb[:, 5, :], corr[:, (j - 1) ^ 1, :],
                             start=False, stop=False)
            nc.tensor.matmul(tps[:, sl1, :], tb_sb[:, 4, :], corr[:, j ^ 1, :],
                             start=False, stop=True)
        if j % 2 == 1:
            sl = slice(j - 1, j + 1)
            nc.vector.tensor_add(out=big_ob[:, sl, :], in0=corr[:, sl, :],
                                 in1=tps[:])
        if j % 4 == 3:
            qn = j // 4
            nc.sync.dma_start(out_d[:, 4 * qn : 4 * qn + 4, :],
                              big_ob[:, 4 * qn : 4 * qn + 4, :])


def build_nc(reps=1):
    nc = bacc.Bacc("TRN2", target_bir_lowering=False, debug=False)
    xr8_d = nc.dram_tensor("xr8", [128, NCHUNK, RPC], _f8, kind="ExternalInput")
    xbf_d = nc.dram_tensor("xbf", [128, NCHUNK, RPC], _bf16, kind="ExternalInput")
    e2_d = nc.dram_tensor("e2", [S, 128, 31, 128], _f8, kind="ExternalInput")
    tb_d = nc.dram_tensor("tb", [128, 6, 128], _bf16, kind="ExternalInput")
    out_d = nc.dram_tensor("out", [128, NCHUNK, RPC], _f32, kind="ExternalOutput")

    with tile.TileContext(nc) as tc:
        with (
            tc.tile_pool(name="const", bufs=1) as constp,
            tc.tile_pool(name="xrp", bufs=2) as xrp,
            tc.tile_pool(name="work", bufs=6) as workp,
            tc.tile_pool(name="psum_a", bufs=5, space="PSUM") as psum_a,
            tc.tile_pool(name="psum_t", bufs=2, space="PSUM") as psum_t,
        ):
            pools = (constp, xrp, workp, psum_a, psum_t)
            if reps == 1:
                _emit_body(nc, pools, xr8_d, xbf_d, e2_d, tb_d, out_d)
            else:
                with tc.For_i(0, reps, 1, staggered_reset=True,
                              hint_engines=(mybir.EngineType.PE,)):
                    _emit_body(nc, pools, xr8_d, xbf_d, e2_d, tb_d, out_d)
    nc.compile()
    return nc


_NC_CACHE = {}


def _get_nc(reps=1):
    if reps not in _NC_CACHE:
        _NC_CACHE[reps] = build_nc(reps)
    return _NC_CACHE[reps]


def make_in_maps(x, diagonals):
    c = _circ_cols(diagonals)
    e2 = (c * WSCALE)[:, _E2F_IDX].astype(_np_f8).reshape(S, 128, 31, 128)
    tb = np.concatenate([_TBAND, -_TBAND], axis=1).astype(_np_bf16)  # [128, 6, 128]
    xT = np.asarray(x, dtype=np.float32).reshape(ROWS, L).T  # [L, ROWS]
    in_maps = []
    for i in range(NCORES):
        xs = xT[:, i * RPC : (i + 1) * RPC].reshape(NCHUNK, 128, RPC)
        xs = np.ascontiguousarray(xs[_PERM].transpose(1, 0, 2))  # [128, 16, RPC]
        in_maps.append({
            "xr8": xs.astype(_np_f8),
            "xbf": xs.astype(_np_bf16),
            "e2": e2,
            "tb": tb,
        })
    return in_maps


def gather_out(results):
    parts = []
    for r in results:
        o = r["out"]  # [128, NCHUNK(position order), RPC]
        parts.append(np.ascontiguousarray(
            o.transpose(1, 0, 2)[_PERM]).reshape(L, RPC))
    outT = np.concatenate(parts, axis=1)  # [L, ROWS]
    return np.ascontiguousarray(outT.T).reshape(B, C, L).astype(np.float32)


def kernel(x, diagonals):
    x = np.asarray(x, dtype=np.float32)
    assert x.shape == (B, C, L) and np.asarray(diagonals).shape == (S, L)
    nc = _get_nc(1)
    in_maps = make_in_maps(x, diagonals)
    last_err = None
    for attempt in range(3):
        try:
            res = run_bass_kernel_spmd(nc, in_maps, core_ids=list(range(NCORES)))
            return gather_out(res.results)
        except Exception as ex:  # transient device errors
            last_err = ex
            import time as _time

            _time.sleep(2.0 * (attempt + 1))
    raise last_err


# revision 10
# speedup vs baseline: 1.8030x; 1.0735x over previous
"""Trainium2 Bass kernel for the seasonal-decomposition block (fp8 DoubleRow).

Math: for each season s, circ_s = real(F_s^H diag(d_s) F_s) is a symmetric
Toeplitz matrix whose first column c_s(t) is computed on host with one
length-N FFT. Every 128x128 block of circ_s is a contiguous 128-column
slice of the skewed buffer E2F_s[p, m] = c_s(|1920 + p - m|) ([128, 3968]),
so the LxL matrix is never materialized.

The recurrence  x_rem <- x_rem - tanh(x_rem @ circ_s)  runs in transposed
layout (positions on partitions, rows on the free axis). Matmuls use fp8
(e4m3) operands with MatmulPerfMode.DoubleRow: each instruction contracts
two adjacent 128-blocks at double rate. Adjacent position-chunks are
pair-swapped in the x layout (position j holds logical chunk j^1) so a
single [128, 2, RPC] access pattern pairs with two adjacent weight blocks
of the skew buffer, which are contiguous by construction.

State: x_rem is never materialized. Using x_rem_s = x - corr_s (with
corr_s = sum_{r<s} tanh_r, kept in bf16), the fp8 PE operand for the next
season is one fused DVE op  xr8 = fp8(x_bf - corr)  per chunk-pair, and
the trailing avg-pool trend is  T.T @ x_bf - T.T @ corr  accumulated in
one PSUM group via positive and negated bf16 band matrices (fp8 bands
would corrupt the replicate-pad coefficients). Weights are scaled by 256
on host before fp8 quantization (entries ~3e-3 would be subnormal) and
descaled inside the tanh activation (scale=1/256). Emulated end-to-end
rel_err vs the fp64 reference: 1.20e-2 (gate: 2e-2).

Sharding: pure data-parallel over the B*C = 2048 rows, 256 rows per core,
8 cores, no collectives.
"""

import sys

sys.path.insert(0, "/opt/trn_rl_repo")

import numpy as np
import ml_dtypes

import concourse.mybir as mybir
import concourse.tile as tile
from concourse import bacc
from concourse.bass_utils import run_bass_kernel_spmd

L = 2048
S = 4
NFULL = L * S
KER = 25
B, C = 64, 32
NCORES = 8
ROWS = B * C          # 2048
RPC = ROWS // NCORES  # 256 rows per core
NCHUNK = L // 128     # 16
WSCALE = 256.0

_f32 = mybir.dt.float32
_f8 = mybir.dt.float8e4
_bf16 = mybir.dt.bfloat16
_np_f8 = ml_dtypes.float8_e4m3
_np_bf16 = ml_dtypes.bfloat16
_DR = mybir.MatmulPerfMode.DoubleRow


def _build_tband():
    """Three [128,128] band blocks of the avg-pool matrix T (trend = T.T @ x)."""
    u = np.arange(128)[:, None]
    t = np.arange(128)[None, :]
    diag = ((t - u >= 0) & (t - u <= KER - 1)).astype(np.float32) / KER
    sub = ((u - t) >= 128 - (KER - 1)).astype(np.float32) / KER
    t00 = diag.copy()
    t00[0, :] += np.maximum(0, (KER - 1) - np.arange(128)).astype(np.float32) / KER
    return np.ascontiguousarray(np.stack([t00, diag, sub], axis=1))  # [128, 3, 128]


_TBAND = _build_tband()
# skew index: E2F[p, m] = c(|1920 + p - m|), block(d) at cols [1920-128d, 2048-128d)
_E2F_IDX = np.abs(1920 + np.arange(128)[:, None] - np.arange(31 * 128)[None, :])
# pair-swap permutation: position j holds logical chunk j^1
_PERM = np.arange(NCHUNK) ^ 1


def _circ_cols(diagonals):
    """First columns c_s(t), t = 0..L-1, of each season's Toeplitz circ_s."""
    d = np.zeros((S, NFULL), dtype=np.float64)
    d[:, :L] = np.asarray(diagonals, dtype=np.float64)
    F = np.fft.fft(d, axis=1)
    t = np.arange(L)
    ph = np.exp((2j * np.pi / NFULL) * (np.arange(S)[:, None] * L * t[None, :]))
    return ((ph * np.conj(F[:, :L])).real / NFULL).astype(np.float32)  # [S, L]


def _emit_body(nc, pools, xr8_d, xbf_d, e2_d, tb_d, out_d):
    constp, xrp, workp, psum_a, psum_t = pools
    tanh_f = mybir.ActivationFunctionType.Tanh

    # ---- SBUF tiles + prologue DMA (ordered by first use) ----
    e2_sb = [constp.tile([128, 31, 128], _f8, tag=f"e2_{s}", name=f"e2_{s}") for s in range(S)]
    xr8_0 = constp.tile([128, NCHUNK, RPC], _f8, tag="xr8_0", name="xr8_0")
    xbf_0 = constp.tile([128, NCHUNK, RPC], _bf16, tag="xbf_0", name="xbf_0")
    tb_sb = constp.tile([128, 6, 128], _bf16, tag="tb")
    corr = constp.tile([128, NCHUNK, RPC], _bf16, tag="corr", name="corr")
    big_ob = constp.tile([128, NCHUNK, RPC], _f32, tag="bigob")

    # season-0 chain b=0 runs k ascending with rhs pair p=7 first
    nc.sync.dma_start(xr8_0[:], xr8_d[:])
    nc.sync.dma_start(e2_sb[0][:, 0:16, :], e2_d[0][:, 0:16, :])
    nc.sync.dma_start(e2_sb[0][:, 16:31, :], e2_d[0][:, 16:31, :])
    nc.sync.dma_start(xbf_0[:], xbf_d[:])
    for s in range(1, S):
        nc.sync.dma_start(e2_sb[s][:], e2_d[s])
    nc.sync.dma_start(tb_sb[:], tb_d[:])

    xr8_cur = xr8_0

    for s in range(S):
        last = s == S - 1
        xr8_nxt = None if last else xrp.tile(
            [128, NCHUNK, RPC], _f8, tag="xr8n", name=f"xr8n{s}")
        for b in range(NCHUNK):
            q = b // 2
            if b % 2 == 0:
                acc = psum_a.tile([128, 2, RPC], _f32, tag="acc")
            # season 0: ascending k (weight DMA arrival order);
            # later seasons: ascending pair (x written pair-by-pair)
            p_order = range(7, -1, -1) if s == 0 else range(8)
            for i, p in enumerate(p_order):
                k = b + 14 - 2 * p
                # chunk b lives at position b^1; within the pair that's 1-b%2
                nc.tensor.matmul(
                    acc[:, 1 - b % 2, :],
                    e2_sb[s][:, k : k + 2, :],
                    xr8_cur[:, 2 * p : 2 * p + 2, :],
                    start=(i == 0),
                    stop=(i == 7),
                    perf_mode=_DR,
                )
            if b % 2 == 1:
                sl = slice(2 * q, 2 * q + 2)
                # season 0: tanh lands straight in corr (corr = t); later
                # seasons: tanh to a scratch pair then corr += t. Then the
                # fused  xr8 = fp8(x - corr)  subcast. All on Act + DVE.
                if s == 0:
                    nc.scalar.activation(corr[:, sl, :], acc[:], tanh_f,
                                         scale=1.0 / WSCALE)
                else:
                    t_pair = workp.tile([128, 2, RPC], _bf16, tag="tp",
                                        name=f"tp{s}_{q}")
                    nc.scalar.activation(t_pair[:], acc[:], tanh_f,
                                         scale=1.0 / WSCALE)
                    nc.vector.tensor_add(out=corr[:, sl, :], in0=corr[:, sl, :],
                                         in1=t_pair[:])
                if not last:
                    nc.vector.tensor_sub(out=xr8_nxt[:, sl, :], in0=xbf_0[:, sl, :],
                                         in1=corr[:, sl, :])
        xr8_cur = xr8_nxt

    # ---- trend = T.T @ (x - corr) in one PSUM group; out = corr + trend ----
    # big_ob is kept in position order (chunk j^1 at index j); host unswaps.
    for j in range(NCHUNK):
        if j % 2 == 0:
            tps = psum_t.tile([128, 2, RPC], _f32, tag="tps", name=f"tps{j//2}")
        sl1 = 1 - j % 2  # position of logical chunk j within its pair
        if j == 0:
            nc.tensor.matmul(tps[:, sl1, :], tb_sb[:, 0, :], xbf_0[:, 1, :],
                             start=True, stop=False)
            nc.tensor.matmul(tps[:, sl1, :], tb_sb[:, 3, :], corr[:, 1, :],
                             start=False, stop=True)
        else:
            nc.tensor.matmul(tps[:, sl1, :], tb_sb[:, 2, :], xbf_0[:, (j - 1) ^ 1, :],
                             start=True, stop=False)
            nc.tensor.matmul(tps[:, sl1, :], tb_sb[:, 1, :], xbf_0[:, j ^ 1, :],
                             start=False, stop=False)
            nc.tensor.matmul(tps[:, sl1, :], tb_sb[:, 5, :], corr[:, (j - 1) ^ 1, :],
                             start=False, stop=False)
            nc.tensor.matmul(tps[:, sl1, :], tb_sb[:, 4, :], corr[:, j ^ 1, :],
                             start=False, stop=True)
        if j % 2 == 1:
            sl = slice(j - 1, j + 1)
            nc.vector.tensor_add(out=big_ob[:, sl, :], in0=corr[:, sl, :],
                                 in1=tps[:])
            nc.sync.dma_start(out_d[:, sl, :], big_ob[:, sl, :])


def build_nc(reps=1):
    nc = bacc.Bacc("TRN2", target_bir_lowering=False, debug=False)
    xr8_d = nc.dram_tensor("xr8", [128, NCHUNK, RPC], _f8, kind="ExternalInput")
    xbf_d = nc.dram_tensor("xbf", [128, NCHUNK, RPC], _bf16, kind="ExternalInput")
    e2_d = nc.dram_tensor("e2", [S, 128, 31, 128], _f8, kind="ExternalInput")
    tb_d = nc.dram_tensor("tb", [128, 6, 128], _bf16, kind="ExternalInput")
    out_d = nc.dram_tensor("out", [128, NCHUNK, RPC], _f32, kind="ExternalOutput")

    with tile.TileContext(nc) as tc:
        with (
            tc.tile_pool(name="const", bufs=1) as constp,
            tc.tile_pool(name="xrp", bufs=2) as xrp,
            tc.tile_pool(name="work", bufs=6) as workp,
            tc.tile_pool(name="psum_a", bufs=5, space="PSUM") as psum_a,
            tc.tile_pool(name="psum_t", bufs=2, space="PSUM") as psum_t,
        ):
            pools = (constp, xrp, workp, psum_a, psum_t)
            if reps == 1:
                _emit_body(nc, pools, xr8_d, xbf_d, e2_d, tb_d, out_d)
            else:
                with tc.For_i(0, reps, 1, staggered_reset=True,
                              hint_engines=(mybir.EngineType.PE,)):
                    _emit_body(nc, pools, xr8_d, xbf_d, e2_d, tb_d, out_d)
    nc.compile()
    return nc


_NC_CACHE = {}


def _get_nc(reps=1):
    if reps not in _NC_CACHE:
        _NC_CACHE[reps] = build_nc(reps)
    return _NC_CACHE[reps]


def make_in_maps(x, diagonals):
    c = _circ_cols(diagonals)
    e2 = (c * WSCALE)[:, _E2F_IDX].astype(_np_f8).reshape(S, 128, 31, 128)
    tb = np.concatenate([_TBAND, -_TBAND], axis=1).astype(_np_bf16)  # [128, 6, 128]
    xT = np.asarray(x, dtype=np.float32).reshape(ROWS, L).T  # [L, ROWS]
    in_maps = []
    for i in range(NCORES):
        xs = xT[:, i * RPC : (i + 1) * RPC].reshape(NCHUNK, 128, RPC)
        xs = np.ascontiguousarray(xs[_PERM].transpose(1, 0, 2))  # [128, 16, RPC]
        in_maps.append({
            "xr8": xs.astype(_np_f8),
            "xbf": xs.astype(_np_bf16),
            "e2": e2,
            "tb": tb,
        })
    return in_maps


def gather_out(results):
    parts = []
    for r in results:
        o = r["out"]  # [128, NCHUNK(position order), RPC]
        parts.append(np.ascontiguousarray(
            o.transpose(1, 0, 2)[_PERM]).reshape(L, RPC))
    outT = np.concatenate(parts, axis=1)  # [L, ROWS]
    return np.ascontiguousarray(outT.T).reshape(B, C, L).astype(np.float32)


def kernel(x, diagonals):
    x = np.asarray(x, dtype=np.float32)
    assert x.shape == (B, C, L) and np.asarray(diagonals).shape == (S, L)
    nc = _get_nc(1)
    in_maps = make_in_maps(x, diagonals)
    last_err = None
    for attempt in range(3):
        try:
            res = run_bass_kernel_spmd(nc, in_maps, core_ids=list(range(NCORES)))
            return gather_out(res.results)
        except Exception as ex:  # transient device errors
            last_err = ex
            import time as _time

            _time.sleep(2.0 * (attempt + 1))
    raise last_err
